# revision 5
# baseline (speedup 1.0000x reference)
"""Trainium2 Bass kernel for nn_LocalRNN: 8-step CTRNN over sliding windows.

Math:
  For each position l: h_{k+1} = a*h_k + relu(h_k @ W* + u*[l+k]),  h_0 = 0
  where a = 1 - 1/tau, W* = W * (1/tau) (columns), u* = Xp @ W_in* + b*,
  W_in* = W_in * (1/tau), b* = b * (1/tau).  Output = h_8 per position.
  (Uses relu(c*z) = c*relu(z) for c>0 to fold 1/tau into the weights, and
  the fact that the input projection is shared across overlapping windows.)

Sharding: batch dim (8) across the 8 NeuronCores, weights replicated.
On-chip layout is transposed ([d on partitions, positions on free dim]) so
matmuls contract d on the partition axis; the host uploads x pre-transposed
in bf16 and transposes the bf16 [d, pos] output back (layout marshalling).

v2: everything bf16 (PE same speed as f32r, but DVE 2-byte fast modes +
half the DMA bytes). Per step the four [128,1024] tiles split work:
  - u-add into PSUM: identity matmul on PE for 3 tiles, one tile goes the
    DVE route (stt z+u then tensor_scalar max) to shave PE columns
  - relu: ACT for 3 tiles (PSUM->SBUF bf16)
  - h-update (h' = a*h + r): DVE tensor_scalar+tensor_tensor (4x/2x modes)
    for 2 tiles, gpsimd scalar_tensor_tensor for 2 tiles
Input x lands via 4 position-quarter DMAs on 4 queues; output leaves bf16.
"""

import numpy as np
import ml_dtypes
from contextlib import ExitStack

import concourse.bass as bass
import concourse.tile as tile
from concourse import bacc, mybir
from concourse.bass_utils import run_bass_kernel_spmd

B, L, D, KSIZE = 8, 2048, 256, 8
P = 128
NCORES = 8
MMN = 512                    # matmul moving free dim (PSUM bank limit)
WCH = 1024                   # tile width for PSUM tiles / elementwise ops
NW = L // WCH                # 2
UCOLS = L + KSIZE - 1        # 2055
PAD = KSIZE - 1              # 7
DB = D // P                  # 2 d-blocks
F32 = mybir.dt.float32
BF16 = mybir.dt.bfloat16
AF = mybir.ActivationFunctionType
ALU = mybir.AluOpType
BF16NP = ml_dtypes.bfloat16

# packed bf16 const blobs: cru = wint0|wint1 ; crw = wt0|wt1|identity
CRU_COLS = 2 * D
CRW_COLS = 2 * D + P
CRW_ID = 2 * D
# packed f32 consts blob: bst | at | pad src
CF_COLS = 2 * DB + PAD + 1
_cache = {}

# --- tuning flags ---
DVE_SIDE_TILE = True      # one tile/step adds u on DVE instead of PE identity
N_POOL_UPDATES = 2        # how many of the 4 per-step h-updates go to gpsimd
N_WARM = 4                # dummy matmuls to engage the PE clock early


def _build_program():
    nc = bacc.Bacc(
        "TRN2",
        target_bir_lowering=False,
        debug=False,
        enable_asserts=False,
        num_devices=NCORES,
    )
    # x uploaded pre-transposed bf16: (D, L), row d -> [d, positions]
    x_d = nc.dram_tensor("xt", (D, L), BF16, kind="ExternalInput").ap()
    cru_d = nc.dram_tensor("constsru", (P, CRU_COLS), BF16, kind="ExternalInput").ap()
    crw_d = nc.dram_tensor("constsrw", (P, CRW_COLS), BF16, kind="ExternalInput").ap()
    cf_d = nc.dram_tensor("constsf", (P, CF_COLS), F32, kind="ExternalInput").ap()
    # output in T-layout bf16: (D, L); host transposes + upcasts
    out_d = nc.dram_tensor("out", (D, L), BF16, kind="ExternalOutput").ap()

    with tile.TileContext(nc) as tc, ExitStack() as ctx:
        consts = ctx.enter_context(tc.tile_pool(name="consts", bufs=1))
        big = ctx.enter_context(tc.tile_pool(name="big", bufs=1))
        rp = ctx.enter_context(tc.tile_pool(name="rp", bufs=4))
        ahp = ctx.enter_context(tc.tile_pool(name="ahp", bufs=3))
        ttp = ctx.enter_context(tc.tile_pool(name="ttp", bufs=2))
        # single PSUM pool: [128,1024] slot = 2 banks, bufs=4 -> all 8 banks
        zp = ctx.enter_context(tc.tile_pool(name="zp", bufs=4, space="PSUM"))

        # --- PE warmup: dummy matmuls on zeros to engage the clock early ---
        dummy = big.tile([P, MMN], BF16, name="dummy")
        nc.gpsimd.memset(dummy[:], 0.0)
        for _ in range(N_WARM):
            warm = zp.tile([P, WCH], F32, name="warm", tag="z")
            nc.tensor.matmul(warm[:, 0:MMN], lhsT=dummy[:, 0:P], rhs=dummy[:],
                             start=True, stop=True)

        # --- constants ---
        cru = consts.tile([P, CRU_COLS], BF16, name="cru")
        crw = consts.tile([P, CRW_COLS], BF16, name="crw")
        cf = consts.tile([P, CF_COLS], F32, name="cf")
        wt = [crw[:, i * D:(i + 1) * D] for i in range(DB)]
        wint = [cru[:, i * D:(i + 1) * D] for i in range(DB)]
        identb = crw[:, CRW_ID:CRW_ID + P]
        bst = cf[:, 0:DB]
        at = cf[:, DB:2 * DB]
        padsrc = cf[:, 2 * DB:2 * DB + PAD]

        # --- persistent buffers (bf16) ---
        # x in 2 per-position-half tiles (both d-blocks each)
        xth = [big.tile([P, DB * WCH], BF16, name=f"xth{g}") for g in range(2)]
        ut = [big.tile([P, UCOLS], BF16, name=f"ut{i}") for i in range(DB)]
        hball = [big.tile([P, DB * L], BF16, name=f"hb{s}") for s in range(2)]
        hb = [[hball[s][:, i * L:(i + 1) * L] for i in range(DB)]
              for s in range(2)]
        h1 = hb[1]

        # --- input DMAs: x in 4 position-quarters on 4 queues; cf + cru
        # (needed by the first u-chunk) land first on their queues.
        def xdma(eng, q):
            g, half = q // 2, q % 2
            eng.dma_start(
                xth[g][:].rearrange("p (i c) -> p i c", i=DB)[
                    :, :, half * MMN:(half + 1) * MMN],
                x_d.rearrange("(i p) c -> p i c", p=P)[
                    :, :, q * MMN:(q + 1) * MMN],
            )
        nc.sync.dma_start(cf[:], cf_d[:, :])
        nc.scalar.dma_start(cru[:], cru_d[:, :])
        xdma(nc.sync, 0)
        xdma(nc.scalar, 1)
        xdma(nc.gpsimd, 2)
        xdma(nc.sync, 3)
        nc.sync.dma_start(crw[:], crw_d[:, :])

        # u pad cols + h1 pad cols (also warms the ACT table early):
        # u[:, :7] = b*, h1[:, :7] = relu(b*)
        for j in range(DB):
            nc.scalar.activation(
                ut[j][:, 0:PAD], padsrc,
                AF.Identity, bias=bst[:, j:j + 1], scale=0.0,
            )
            nc.scalar.activation(
                h1[j][:, 0:PAD], padsrc,
                AF.Relu, bias=bst[:, j:j + 1], scale=0.0,
            )

        # --- u projection: 4 tiles (gw, j); i-outer order shares LDWEIGHTS
        # across the two 512 halves. Post ops split between ACT and DVE.
        for gw in range(2):
            for j in range(DB):
                zt = zp.tile([P, WCH], F32, name="zu", tag="z")
                for i in range(DB):
                    for half in range(2):
                        nc.tensor.matmul(
                            zt[:, half * MMN:(half + 1) * MMN],
                            lhsT=wint[i][:, j * P:(j + 1) * P],
                            rhs=xth[gw][:, i * WCH + half * MMN:
                                        i * WCH + half * MMN + MMN],
                            start=(i == 0),
                            stop=(i == DB - 1),
                        )
                # u positions [7+1024gw, 7+1024(gw+1)), h1 same minus tail
                us = PAD + gw * WCH
                hw = WCH if gw == 0 else WCH - PAD
                if gw == 0:
                    nc.scalar.activation(
                        ut[j][:, us:us + WCH], zt[:],
                        AF.Identity, bias=bst[:, j:j + 1], scale=1.0,
                    )
                    nc.vector.tensor_scalar(
                        out=h1[j][:, us:us + hw], in0=zt[:, 0:hw],
                        scalar1=bst[:, j:j + 1], scalar2=0.0,
                        op0=ALU.add, op1=ALU.max,
                    )
                else:
                    nc.vector.tensor_scalar(
                        out=ut[j][:, us:us + WCH], in0=zt[:],
                        scalar1=bst[:, j:j + 1], scalar2=None,
                        op0=ALU.add,
                    )
                    nc.scalar.activation(
                        h1[j][:, us:us + hw], zt[:, 0:hw],
                        AF.Relu, bias=bst[:, j:j + 1], scale=1.0,
                    )

        # --- steps 1..7 ---
        for k in range(1, KSIZE):
            hc = hb[k % 2]
            hn = hb[(k + 1) % 2]
            tix = 0
            for c in range(NW):
                cs = c * WCH
                for j in range(DB):
                    dve_side = DVE_SIDE_TILE and (c, j) == (0, 1)
                    zt = zp.tile([P, WCH], F32, name="zt", tag="z")
                    if not dve_side:
                        # identity matmul first (u ready early), W blocks after
                        for half in range(2):
                            nc.tensor.matmul(
                                zt[:, half * MMN:(half + 1) * MMN],
                                lhsT=identb,
                                rhs=ut[j][:, k + cs + half * MMN:
                                          k + cs + half * MMN + MMN],
                                start=True, stop=False,
                            )
                    for i in range(DB):
                        last = (i == DB - 1)
                        for half in range(2):
                            hs = cs + half * MMN
                            nc.tensor.matmul(
                                zt[:, half * MMN:(half + 1) * MMN],
                                lhsT=wt[i][:, j * P:(j + 1) * P],
                                rhs=hc[i][:, hs:hs + MMN],
                                start=(dve_side and i == 0),
                                stop=last,
                            )
                    r = rp.tile([P, WCH], BF16, name="r", tag="r")
                    if dve_side:
                        # z+u on DVE (PSUM read), then relu via 2-byte TS max
                        t = ttp.tile([P, WCH], BF16, name="t", tag="t")
                        nc.vector.scalar_tensor_tensor(
                            out=t[:], in0=zt[:], scalar=0.0,
                            in1=ut[j][:, k + cs:k + cs + WCH],
                            op0=ALU.add, op1=ALU.add,
                        )
                        nc.vector.tensor_scalar(
                            out=r[:], in0=t[:], scalar1=0.0, scalar2=None,
                            op0=ALU.max,
                        )
                    else:
                        nc.scalar.activation(r[:], zt[:], AF.Relu)
                    # h update: h' = a*h + r. DVE pre-scales (4x bf16 mode);
                    # the add runs on gpsimd for some tiles to offload DVE.
                    ah = ahp.tile([P, WCH], BF16, name="ah", tag="ah")
                    nc.vector.tensor_scalar(
                        out=ah[:], in0=hc[j][:, cs:cs + WCH],
                        scalar1=at[:, j:j + 1], scalar2=None,
                        op0=ALU.mult,
                    )
                    add_eng = nc.gpsimd if tix < N_POOL_UPDATES else nc.vector
                    add_eng.tensor_tensor(
                        hn[j][:, cs:cs + WCH], ah[:], r[:], ALU.add,
                    )
                    tix += 1
                # output DMA per 1024-chunk after the last step
                if k == KSIZE - 1:
                    h8all = hball[(k + 1) % 2]
                    eng = nc.sync if c == 0 else nc.gpsimd
                    eng.dma_start(
                        out_d.rearrange("(i p) c -> p i c", p=P)[
                            :, :, cs:cs + WCH],
                        h8all[:].rearrange("p (i c) -> p i c", i=DB)[
                            :, :, cs:cs + WCH],
                    )

    nc.compile()
    return nc


def get_program():
    if "nc" not in _cache:
        _cache["nc"] = _build_program()
    return _cache["nc"]


def make_in_maps(x, weight, input_weight, bias, tau):
    x = np.asarray(x, dtype=np.float32)
    weight = np.asarray(weight, dtype=np.float32)
    input_weight = np.asarray(input_weight, dtype=np.float32)
    bias = np.asarray(bias, dtype=np.float32).reshape(1, D)
    tau = np.asarray(tau, dtype=np.float32).reshape(1, D)

    inv_tau = 1.0 / tau                       # (1, D)
    a = 1.0 - inv_tau
    wstar = (weight * inv_tau).astype(np.float32)          # scale columns
    winstar = (input_weight * inv_tau).astype(np.float32)
    bstar = (bias * inv_tau).astype(np.float32)
    # per-partition layout (P, DB): col j holds elems [j*P, (j+1)*P)
    bstar_t = bstar.reshape(DB, P).T
    a_t = a.reshape(DB, P).T
    ident = np.eye(P, dtype=np.float32)

    cru = np.concatenate([winstar[0:P, :], winstar[P:D, :]], axis=1)
    crw = np.concatenate([wstar[0:P, :], wstar[P:D, :], ident], axis=1)
    cf = np.concatenate(
        [bstar_t, a_t, np.zeros((P, PAD + 1), np.float32)], axis=1)

    shared = {
        "constsru": np.ascontiguousarray(cru.astype(BF16NP)),
        "constsrw": np.ascontiguousarray(crw.astype(BF16NP)),
        "constsf": np.ascontiguousarray(cf),
    }
    return [
        {"xt": np.ascontiguousarray(x[b].T.astype(BF16NP)), **shared}
        for b in range(NCORES)
    ]


def kernel(x, weight, input_weight, bias, tau, ksize, _trace=False):
    assert int(ksize) == KSIZE
    nc = get_program()
    in_maps = make_in_maps(x, weight, input_weight, bias, tau)
    res = run_bass_kernel_spmd(
        nc, in_maps, core_ids=list(range(NCORES)), trace=_trace
    )
    out = np.stack(
        [np.ascontiguousarray(res.results[b]["out"].T) for b in range(NCORES)],
        axis=0,
    )
    if _trace:
        _cache["last_results"] = res
    return out.astype(np.float32)


# revision 9
# speedup vs baseline: 1.1543x; 1.1543x over previous
"""Trainium2 Bass kernel for nn_LocalRNN: 8-step CTRNN over sliding windows.

Math:
  For each position l: h_{k+1} = a*h_k + relu(h_k @ W* + u*[l+k]),  h_0 = 0
  where a = 1 - 1/tau, W* = W * (1/tau) (columns), u* = Xp @ W_in* + b*,
  W_in* = W_in * (1/tau), b* = b * (1/tau).  Output = h_8 per position.
  (Uses relu(c*z) = c*relu(z) for c>0 to fold 1/tau into the weights, and
  the fact that the input projection is shared across overlapping windows.)

Sharding: batch dim (8) across the 8 NeuronCores, weights replicated.
On-chip layout is transposed ([d on partitions, positions on free dim]) so
matmuls contract d on the partition axis; the host uploads x pre-transposed
in bf16 and transposes the bf16 [d, pos] output back (layout marshalling).

v2: everything bf16 (PE same speed as f32r, but DVE 2-byte fast modes +
half the DMA bytes). Per step the four [128,1024] tiles split work:
  - u-add into PSUM: identity matmul on PE for 3 tiles, one tile goes the
    DVE route (stt z+u then tensor_scalar max) to shave PE columns
  - relu: ACT for 3 tiles (PSUM->SBUF bf16)
  - h-update (h' = a*h + r): DVE tensor_scalar+tensor_tensor (4x/2x modes)
    for 2 tiles, gpsimd scalar_tensor_tensor for 2 tiles
Input x lands via 4 position-quarter DMAs on 4 queues; output leaves bf16.
"""

import numpy as np
import ml_dtypes
from contextlib import ExitStack

import concourse.bass as bass
import concourse.tile as tile
from concourse import bacc, mybir
from concourse.bass_utils import run_bass_kernel_spmd

B, L, D, KSIZE = 8, 2048, 256, 8
P = 128
NCORES = 8
MMN = 512                    # matmul moving free dim (PSUM bank limit)
WCH = 1024                   # tile width for PSUM tiles / elementwise ops
NW = L // WCH                # 2
UCOLS = L + KSIZE - 1        # 2055
PAD = KSIZE - 1              # 7
DB = D // P                  # 2 d-blocks
F32 = mybir.dt.float32
BF16 = mybir.dt.bfloat16
AF = mybir.ActivationFunctionType
ALU = mybir.AluOpType
BF16NP = ml_dtypes.bfloat16

# packed bf16 const blobs: cru = wint0|wint1 ; crw = wt0|wt1|identity
CRU_COLS = 2 * D
CRW_COLS = 2 * D + P
CRW_ID = 2 * D
# packed f32 consts blob: bst | at | pad src
CF_COLS = 2 * DB + PAD + 1
_cache = {}

# --- tuning flags ---
DVE_SIDE_TILE = False     # one tile/step adds u on DVE instead of PE identity
POOL_TILE = 1             # tile index (c*DB+j) whose h-update add runs on gpsimd
N_WARM = 8                # dummy matmuls to engage the PE clock early


def _build_program():
    nc = bacc.Bacc(
        "TRN2",
        target_bir_lowering=False,
        debug=False,
        enable_asserts=False,
        num_devices=NCORES,
    )
    # x uploaded pre-transposed bf16: (D, L), row d -> [d, positions]
    x_d = nc.dram_tensor("xt", (D, L), BF16, kind="ExternalInput").ap()
    cru_d = nc.dram_tensor("constsru", (P, CRU_COLS), BF16, kind="ExternalInput").ap()
    crw_d = nc.dram_tensor("constsrw", (P, CRW_COLS), BF16, kind="ExternalInput").ap()
    cf_d = nc.dram_tensor("constsf", (P, CF_COLS), F32, kind="ExternalInput").ap()
    # output in T-layout bf16: (D, L); host transposes + upcasts
    out_d = nc.dram_tensor("out", (D, L), BF16, kind="ExternalOutput").ap()

    with tile.TileContext(nc) as tc, ExitStack() as ctx:
        consts = ctx.enter_context(tc.tile_pool(name="consts", bufs=1))
        big = ctx.enter_context(tc.tile_pool(name="big", bufs=1))
        rp = ctx.enter_context(tc.tile_pool(name="rp", bufs=4))
        ahp = ctx.enter_context(tc.tile_pool(name="ahp", bufs=3))
        ttp = ctx.enter_context(tc.tile_pool(name="ttp", bufs=2))
        # single PSUM pool: [128,1024] slot = 2 banks, bufs=4 -> all 8 banks
        zp = ctx.enter_context(tc.tile_pool(name="zp", bufs=4, space="PSUM"))

        # --- PE warmup: dummy matmuls on zeros to engage the clock early ---
        dummy = big.tile([P, MMN], BF16, name="dummy")
        nc.gpsimd.memset(dummy[:], 0.0)
        for _ in range(N_WARM):
            warm = zp.tile([P, WCH], F32, name="warm", tag="z")
            nc.tensor.matmul(warm[:, 0:MMN], lhsT=dummy[:, 0:P], rhs=dummy[:],
                             start=True, stop=True)

        # --- constants ---
        cru = consts.tile([P, CRU_COLS], BF16, name="cru")
        crw = consts.tile([P, CRW_COLS], BF16, name="crw")
        cf = consts.tile([P, CF_COLS], F32, name="cf")
        wt = [crw[:, i * D:(i + 1) * D] for i in range(DB)]
        wint = [cru[:, i * D:(i + 1) * D] for i in range(DB)]
        identb = crw[:, CRW_ID:CRW_ID + P]
        bst = cf[:, 0:DB]
        at = cf[:, DB:2 * DB]
        padsrc = cf[:, 2 * DB:2 * DB + PAD]

        # --- persistent buffers (bf16) ---
        # x in 2 per-position-half tiles (both d-blocks each)
        xth = [big.tile([P, DB * WCH], BF16, name=f"xth{g}") for g in range(2)]
        ut = [big.tile([P, UCOLS], BF16, name=f"ut{i}") for i in range(DB)]
        hball = [big.tile([P, DB * L], BF16, name=f"hb{s}") for s in range(2)]
        hb = [[hball[s][:, i * L:(i + 1) * L] for i in range(DB)]
              for s in range(2)]
        h1 = hb[1]

        # --- input DMAs: x in 4 position-quarters on 4 queues; cf + cru
        # (needed by the first u-chunk) land first on their queues.
        def xdma(eng, q):
            g, half = q // 2, q % 2
            eng.dma_start(
                xth[g][:].rearrange("p (i c) -> p i c", i=DB)[
                    :, :, half * MMN:(half + 1) * MMN],
                x_d.rearrange("(i p) c -> p i c", p=P)[
                    :, :, q * MMN:(q + 1) * MMN],
            )
        nc.sync.dma_start(cf[:], cf_d[:, :])
        nc.scalar.dma_start(cru[:], cru_d[:, :])
        xdma(nc.sync, 0)
        xdma(nc.scalar, 1)
        xdma(nc.gpsimd, 2)
        xdma(nc.gpsimd, 3)
        nc.sync.dma_start(crw[:], crw_d[:, :])

        # u pad cols + h1 pad cols (also warms the ACT table early):
        # u[:, :7] = b*, h1[:, :7] = relu(b*)
        for j in range(DB):
            nc.scalar.activation(
                ut[j][:, 0:PAD], padsrc,
                AF.Identity, bias=bst[:, j:j + 1], scale=0.0,
            )
            nc.scalar.activation(
                h1[j][:, 0:PAD], padsrc,
                AF.Relu, bias=bst[:, j:j + 1], scale=0.0,
            )

        # --- u projection: 4 tiles (gw, j); i-outer order shares LDWEIGHTS
        # across the two 512 halves. Post ops split between ACT and DVE.
        for gw in range(2):
            for j in range(DB):
                zt = zp.tile([P, WCH], F32, name="zu", tag="z")
                for i in range(DB):
                    for half in range(2):
                        nc.tensor.matmul(
                            zt[:, half * MMN:(half + 1) * MMN],
                            lhsT=wint[i][:, j * P:(j + 1) * P],
                            rhs=xth[gw][:, i * WCH + half * MMN:
                                        i * WCH + half * MMN + MMN],
                            start=(i == 0),
                            stop=(i == DB - 1),
                        )
                # u positions [7+1024gw, 7+1024(gw+1)), h1 same minus tail
                us = PAD + gw * WCH
                hw = WCH if gw == 0 else WCH - PAD
                if gw == 0:
                    nc.scalar.activation(
                        ut[j][:, us:us + WCH], zt[:],
                        AF.Identity, bias=bst[:, j:j + 1], scale=1.0,
                    )
                    nc.vector.tensor_scalar(
                        out=h1[j][:, us:us + hw], in0=zt[:, 0:hw],
                        scalar1=bst[:, j:j + 1], scalar2=0.0,
                        op0=ALU.add, op1=ALU.max,
                    )
                else:
                    nc.vector.tensor_scalar(
                        out=ut[j][:, us:us + WCH], in0=zt[:],
                        scalar1=bst[:, j:j + 1], scalar2=None,
                        op0=ALU.add,
                    )
                    nc.scalar.activation(
                        h1[j][:, us:us + hw], zt[:, 0:hw],
                        AF.Relu, bias=bst[:, j:j + 1], scale=1.0,
                    )

        # --- steps 1..7 ---
        for k in range(1, KSIZE):
            hc = hb[k % 2]
            hn = hb[(k + 1) % 2]
            tix = 0
            for c in range(NW):
                cs = c * WCH
                for j in range(DB):
                    dve_side = DVE_SIDE_TILE and (c, j) == (0, 1)
                    zt = zp.tile([P, WCH], F32, name="zt", tag="z")
                    if not dve_side:
                        # identity matmul first (u ready early), W blocks after
                        for half in range(2):
                            nc.tensor.matmul(
                                zt[:, half * MMN:(half + 1) * MMN],
                                lhsT=identb,
                                rhs=ut[j][:, k + cs + half * MMN:
                                          k + cs + half * MMN + MMN],
                                start=True, stop=False,
                            )
                    for i in range(DB):
                        last = (i == DB - 1)
                        for half in range(2):
                            hs = cs + half * MMN
                            nc.tensor.matmul(
                                zt[:, half * MMN:(half + 1) * MMN],
                                lhsT=wt[i][:, j * P:(j + 1) * P],
                                rhs=hc[i][:, hs:hs + MMN],
                                start=(dve_side and i == 0),
                                stop=last,
                            )
                    r = rp.tile([P, WCH], BF16, name="r", tag="r")
                    if dve_side:
                        # z+u on DVE (PSUM read), then relu via 2-byte TS max
                        t = ttp.tile([P, WCH], BF16, name="t", tag="t")
                        nc.vector.scalar_tensor_tensor(
                            out=t[:], in0=zt[:], scalar=0.0,
                            in1=ut[j][:, k + cs:k + cs + WCH],
                            op0=ALU.add, op1=ALU.add,
                        )
                        nc.vector.tensor_scalar(
                            out=r[:], in0=t[:], scalar1=0.0, scalar2=None,
                            op0=ALU.max,
                        )
                    else:
                        nc.scalar.activation(r[:], zt[:], AF.Relu)
                    # h update: h' = a*h + r. One early tile goes via DVE
                    # pre-scale + gpsimd add (its 2.2us latency hides inside
                    # the step); the rest are single DVE stt ops.
                    if tix == POOL_TILE:
                        ah = ahp.tile([P, WCH], BF16, name="ah", tag="ah")
                        nc.vector.tensor_scalar(
                            out=ah[:], in0=hc[j][:, cs:cs + WCH],
                            scalar1=at[:, j:j + 1], scalar2=None,
                            op0=ALU.mult,
                        )
                        nc.gpsimd.tensor_tensor(
                            hn[j][:, cs:cs + WCH], ah[:], r[:], ALU.add,
                        )
                    else:
                        nc.vector.scalar_tensor_tensor(
                            out=hn[j][:, cs:cs + WCH],
                            in0=hc[j][:, cs:cs + WCH],
                            scalar=at[:, j:j + 1],
                            in1=r[:],
                            op0=ALU.mult, op1=ALU.add,
                        )
                    tix += 1
                # output DMA per 1024-chunk after the last step
                if k == KSIZE - 1:
                    h8all = hball[(k + 1) % 2]
                    if c == 0:
                        pieces = [(nc.sync, cs, WCH)]
                    else:
                        # split the last chunk across two rings to halve the
                        # trailing transfer under 8-core HBM contention
                        pieces = [(nc.sync, cs, MMN),
                                  (nc.gpsimd, cs + MMN, MMN)]
                    for eng, ps, pw in pieces:
                        eng.dma_start(
                            out_d.rearrange("(i p) c -> p i c", p=P)[
                                :, :, ps:ps + pw],
                            h8all[:].rearrange("p (i c) -> p i c", i=DB)[
                                :, :, ps:ps + pw],
                        )

    nc.compile()
    return nc


def get_program():
    if "nc" not in _cache:
        _cache["nc"] = _build_program()
    return _cache["nc"]


def make_in_maps(x, weight, input_weight, bias, tau):
    x = np.asarray(x, dtype=np.float32)
    weight = np.asarray(weight, dtype=np.float32)
    input_weight = np.asarray(input_weight, dtype=np.float32)
    bias = np.asarray(bias, dtype=np.float32).reshape(1, D)
    tau = np.asarray(tau, dtype=np.float32).reshape(1, D)

    inv_tau = 1.0 / tau                       # (1, D)
    a = 1.0 - inv_tau
    wstar = (weight * inv_tau).astype(np.float32)          # scale columns
    winstar = (input_weight * inv_tau).astype(np.float32)
    bstar = (bias * inv_tau).astype(np.float32)
    # per-partition layout (P, DB): col j holds elems [j*P, (j+1)*P)
    bstar_t = bstar.reshape(DB, P).T
    a_t = a.reshape(DB, P).T
    ident = np.eye(P, dtype=np.float32)

    cru = np.concatenate([winstar[0:P, :], winstar[P:D, :]], axis=1)
    crw = np.concatenate([wstar[0:P, :], wstar[P:D, :], ident], axis=1)
    cf = np.concatenate(
        [bstar_t, a_t, np.zeros((P, PAD + 1), np.float32)], axis=1)

    shared = {
        "constsru": np.ascontiguousarray(cru.astype(BF16NP)),
        "constsrw": np.ascontiguousarray(crw.astype(BF16NP)),
        "constsf": np.ascontiguousarray(cf),
    }
    return [
        {"xt": np.ascontiguousarray(x[b].T.astype(BF16NP)), **shared}
        for b in range(NCORES)
    ]


def kernel(x, weight, input_weight, bias, tau, ksize, _trace=False):
    assert int(ksize) == KSIZE
    nc = get_program()
    in_maps = make_in_maps(x, weight, input_weight, bias, tau)
    res = run_bass_kernel_spmd(
        nc, in_maps, core_ids=list(range(NCORES)), trace=_trace
    )
    out = np.stack(
        [np.ascontiguousarray(res.results[b]["out"].T) for b in range(NCORES)],
        axis=0,
    )
    if _trace:
        _cache["last_results"] = res
    return out.astype(np.float32)


# revision 13
# speedup vs baseline: 1.2736x; 1.1033x over previous
"""Trainium2 Bass kernel for nn_LocalRNN: 8-step CTRNN over sliding windows.

Math:
  For each position l: h_{k+1} = a*h_k + relu(h_k @ W* + u*[l+k]),  h_0 = 0
  where a = 1 - 1/tau, W* = W * (1/tau) (columns), u* = Xp @ W_in* + b*,
  W_in* = W_in * (1/tau), b* = b * (1/tau).  Output = h_8 per position.
  (Uses relu(c*z) = c*relu(z) for c>0 to fold 1/tau into the weights, and
  the fact that the input projection is shared across overlapping windows.)

Sharding: batch dim (8) across the 8 NeuronCores, weights replicated.
On-chip layout is transposed ([d on partitions, positions on free dim]) so
matmuls contract d on the partition axis; the host uploads x pre-transposed
in bf16 and transposes the bf16 [d, pos] output back (layout marshalling).

v2: everything bf16 (PE same speed as f32r, but DVE 2-byte fast modes +
half the DMA bytes). Per step the four [128,1024] tiles split work:
  - u-add into PSUM: identity matmul on PE for 3 tiles, one tile goes the
    DVE route (stt z+u then tensor_scalar max) to shave PE columns
  - relu: ACT for 3 tiles (PSUM->SBUF bf16)
  - h-update (h' = a*h + r): DVE tensor_scalar+tensor_tensor (4x/2x modes)
    for 2 tiles, gpsimd scalar_tensor_tensor for 2 tiles
Input x lands via 4 position-quarter DMAs on 4 queues; output leaves bf16.
"""

import numpy as np
import ml_dtypes
from contextlib import ExitStack

import concourse.bass as bass
import concourse.tile as tile
from concourse import bacc, mybir
from concourse.bass_utils import run_bass_kernel_spmd

B, L, D, KSIZE = 8, 2048, 256, 8
P = 128
NCORES = 8
MMN = 512                    # matmul moving free dim (PSUM bank limit)
WCH = 1024                   # tile width for PSUM tiles / elementwise ops
NW = L // WCH                # 2
UCOLS = L + KSIZE - 1        # 2055
PAD = KSIZE - 1              # 7
DB = D // P                  # 2 d-blocks
F32 = mybir.dt.float32
BF16 = mybir.dt.bfloat16
AF = mybir.ActivationFunctionType
ALU = mybir.AluOpType
BF16NP = ml_dtypes.bfloat16

# packed bf16 const blobs: cru = wint0|wint1 ; crw = wt0|wt1|identity
CRU_COLS = 2 * D
CRW_COLS = 2 * D + P
CRW_ID = 2 * D
# packed f32 consts blob: bst | at | pad src
CF_COLS = 2 * DB + PAD + 1
_cache = {}

# --- tuning flags ---
DVE_SIDE_TILE = False     # one tile/step adds u on DVE instead of PE identity
N_WARM = 6                # dummy matmuls to bridge the PE clock to x-arrival
NXP = 8                   # x DMA pieces (256 cols each) across the 3 rings


def _build_program():
    nc = bacc.Bacc(
        "TRN2",
        target_bir_lowering=False,
        debug=False,
        enable_asserts=False,
        num_devices=NCORES,
    )
    # x uploaded pre-transposed bf16: (D, L), row d -> [d, positions]
    x_d = nc.dram_tensor("xt", (D, L), BF16, kind="ExternalInput").ap()
    cru_d = nc.dram_tensor("constsru", (P, CRU_COLS), BF16, kind="ExternalInput").ap()
    crw_d = nc.dram_tensor("constsrw", (P, CRW_COLS), BF16, kind="ExternalInput").ap()
    cf_d = nc.dram_tensor("constsf", (P, CF_COLS), F32, kind="ExternalInput").ap()
    # output in T-layout bf16: (D, L); host transposes + upcasts
    out_d = nc.dram_tensor("out", (D, L), BF16, kind="ExternalOutput").ap()

    with tile.TileContext(nc) as tc, ExitStack() as ctx:
        consts = ctx.enter_context(tc.tile_pool(name="consts", bufs=1))
        big = ctx.enter_context(tc.tile_pool(name="big", bufs=1))
        rp = ctx.enter_context(tc.tile_pool(name="rp", bufs=4))
        ahp = ctx.enter_context(tc.tile_pool(name="ahp", bufs=3))
        # single PSUM pool: [128,1024] slot = 2 banks, bufs=4 -> all 8 banks
        zp = ctx.enter_context(tc.tile_pool(name="zp", bufs=4, space="PSUM"))

        # --- PE warmup: dummy matmuls on zeros to engage the clock early ---
        dummy = big.tile([P, MMN], BF16, name="dummy")
        nc.gpsimd.memset(dummy[:], 0.0)
        for _ in range(N_WARM):
            warm = zp.tile([P, WCH], F32, name="warm", tag="z")
            nc.tensor.matmul(warm[:, 0:MMN], lhsT=dummy[:, 0:P], rhs=dummy[:],
                             start=True, stop=True)

        # --- constants ---
        cru = consts.tile([P, CRU_COLS], BF16, name="cru")
        crw = consts.tile([P, CRW_COLS], BF16, name="crw")
        cf = consts.tile([P, CF_COLS], F32, name="cf")
        wt = [crw[:, i * D:(i + 1) * D] for i in range(DB)]
        wint = [cru[:, i * D:(i + 1) * D] for i in range(DB)]
        identb = crw[:, CRW_ID:CRW_ID + P]
        bst = cf[:, 0:DB]
        at = cf[:, DB:2 * DB]
        padsrc = cf[:, 2 * DB:2 * DB + PAD]

        # --- persistent buffers (bf16) ---
        # x in 2 per-position-half tiles (both d-blocks each)
        xth = [big.tile([P, DB * WCH], BF16, name=f"xth{g}") for g in range(2)]
        ut = [big.tile([P, UCOLS], BF16, name=f"ut{i}") for i in range(DB)]
        hball = [big.tile([P, DB * L], BF16, name=f"hb{s}") for s in range(2)]
        hb = [[hball[s][:, i * L:(i + 1) * L] for i in range(DB)]
              for s in range(2)]
        h1 = hb[1]

        # --- input DMAs: x in NXP position-pieces round-robined over the 3
        # DMA rings, earliest positions first, so the u-projection can start
        # the moment the first pieces land. cf + cru lead their rings.
        PW = L // NXP
        def xdma(eng, q):
            g, off = (q * PW) // WCH, (q * PW) % WCH
            eng.dma_start(
                xth[g][:].rearrange("p (i c) -> p i c", i=DB)[
                    :, :, off:off + PW],
                x_d.rearrange("(i p) c -> p i c", p=P)[
                    :, :, q * PW:(q + 1) * PW],
            )
        nc.sync.dma_start(cf[:], cf_d[:, :])
        nc.scalar.dma_start(cru[:], cru_d[:, :])
        rings = [nc.sync, nc.scalar, nc.gpsimd]
        for q in range(NXP):
            xdma(rings[q % 3], q)
        nc.gpsimd.dma_start(crw[:], crw_d[:, :])

        # u pad cols + h1 pad cols (also warms the ACT table early):
        # u[:, :7] = b*, h1[:, :7] = relu(b*)
        for j in range(DB):
            nc.scalar.activation(
                ut[j][:, 0:PAD], padsrc,
                AF.Identity, bias=bst[:, j:j + 1], scale=0.0,
            )
            nc.scalar.activation(
                h1[j][:, 0:PAD], padsrc,
                AF.Relu, bias=bst[:, j:j + 1], scale=0.0,
            )

        # --- u projection: 4 tiles (gw, j); i-outer order shares LDWEIGHTS
        # across the two 512 halves. Post ops split between ACT and DVE.
        for gw in range(2):
            for j in range(DB):
                zt = zp.tile([P, WCH], F32, name="zu", tag="z")
                for i in range(DB):
                    for half in range(2):
                        nc.tensor.matmul(
                            zt[:, half * MMN:(half + 1) * MMN],
                            lhsT=wint[i][:, j * P:(j + 1) * P],
                            rhs=xth[gw][:, i * WCH + half * MMN:
                                        i * WCH + half * MMN + MMN],
                            start=(i == 0),
                            stop=(i == DB - 1),
                        )
                # u positions [7+1024gw, 7+1024(gw+1)), h1 same minus tail
                us = PAD + gw * WCH
                hw = WCH if gw == 0 else WCH - PAD
                if gw == 0:
                    nc.scalar.activation(
                        ut[j][:, us:us + WCH], zt[:],
                        AF.Identity, bias=bst[:, j:j + 1], scale=1.0,
                    )
                    nc.vector.tensor_scalar(
                        out=h1[j][:, us:us + hw], in0=zt[:, 0:hw],
                        scalar1=bst[:, j:j + 1], scalar2=0.0,
                        op0=ALU.add, op1=ALU.max,
                    )
                else:
                    nc.vector.tensor_scalar(
                        out=ut[j][:, us:us + WCH], in0=zt[:],
                        scalar1=bst[:, j:j + 1], scalar2=None,
                        op0=ALU.add,
                    )
                    nc.scalar.activation(
                        h1[j][:, us:us + hw], zt[:, 0:hw],
                        AF.Relu, bias=bst[:, j:j + 1], scale=1.0,
                    )

        # --- steps 1..7 ---
        for k in range(1, KSIZE):
            hc = hb[k % 2]
            hn = hb[(k + 1) % 2]
            tix = 0
            for c in range(NW):
                cs = c * WCH
                for j in range(DB):
                    dve_side = DVE_SIDE_TILE and (c, j) == (0, 1)
                    zt = zp.tile([P, WCH], F32, name="zt", tag="z")
                    if not dve_side:
                        # identity matmul first (u ready early), W blocks after
                        for half in range(2):
                            nc.tensor.matmul(
                                zt[:, half * MMN:(half + 1) * MMN],
                                lhsT=identb,
                                rhs=ut[j][:, k + cs + half * MMN:
                                          k + cs + half * MMN + MMN],
                                start=True, stop=False,
                            )
                    for i in range(DB):
                        last = (i == DB - 1)
                        for half in range(2):
                            hs = cs + half * MMN
                            nc.tensor.matmul(
                                zt[:, half * MMN:(half + 1) * MMN],
                                lhsT=wt[i][:, j * P:(j + 1) * P],
                                rhs=hc[i][:, hs:hs + MMN],
                                start=(dve_side and i == 0),
                                stop=last,
                            )
                    # relu + h-update (h' = a*h + r): the a-scale TS only
                    # needs the previous h so it runs early; only the TT add
                    # (0.7us) sits after the relu on the critical path. The
                    # very last chunk (step 7, c1) runs at 512 granularity to
                    # shorten the drain into the output DMA.
                    fine = (k == KSIZE - 1 and c == NW - 1)
                    nsub = 2 if fine else 1
                    sw = WCH // nsub
                    ah = ahp.tile([P, WCH], BF16, name="ah", tag="ah")
                    nc.vector.tensor_scalar(
                        out=ah[:], in0=hc[j][:, cs:cs + WCH],
                        scalar1=at[:, j:j + 1], scalar2=None,
                        op0=ALU.mult,
                    )
                    for s in range(nsub):
                        ss = s * sw
                        r = rp.tile([P, sw], BF16, name="r", tag="r")
                        nc.scalar.activation(r[:], zt[:, ss:ss + sw], AF.Relu)
                        nc.vector.tensor_tensor(
                            hn[j][:, cs + ss:cs + ss + sw],
                            ah[:, ss:ss + sw], r[:], ALU.add,
                        )
                    tix += 1
                # output DMAs after the last step; final 512 split across
                # both rings to halve the trailing transfer
                if k == KSIZE - 1:
                    h8all = hball[(k + 1) % 2]
                    if c == 0:
                        pieces = [(nc.sync, cs, WCH)]
                    else:
                        hq = MMN // 2
                        pieces = [(nc.gpsimd, cs, MMN),
                                  (nc.sync, cs + MMN, hq),
                                  (nc.gpsimd, cs + MMN + hq, hq)]
                    for eng, ps, pw in pieces:
                        eng.dma_start(
                            out_d.rearrange("(i p) c -> p i c", p=P)[
                                :, :, ps:ps + pw],
                            h8all[:].rearrange("p (i c) -> p i c", i=DB)[
                                :, :, ps:ps + pw],
                        )

    nc.compile()
    return nc


def get_program():
    if "nc" not in _cache:
        _cache["nc"] = _build_program()
    return _cache["nc"]


def make_in_maps(x, weight, input_weight, bias, tau):
    x = np.asarray(x, dtype=np.float32)
    weight = np.asarray(weight, dtype=np.float32)
    input_weight = np.asarray(input_weight, dtype=np.float32)
    bias = np.asarray(bias, dtype=np.float32).reshape(1, D)
    tau = np.asarray(tau, dtype=np.float32).reshape(1, D)

    inv_tau = 1.0 / tau                       # (1, D)
    a = 1.0 - inv_tau
    wstar = (weight * inv_tau).astype(np.float32)          # scale columns
    winstar = (input_weight * inv_tau).astype(np.float32)
    bstar = (bias * inv_tau).astype(np.float32)
    # per-partition layout (P, DB): col j holds elems [j*P, (j+1)*P)
    bstar_t = bstar.reshape(DB, P).T
    a_t = a.reshape(DB, P).T
    ident = np.eye(P, dtype=np.float32)

    cru = np.concatenate([winstar[0:P, :], winstar[P:D, :]], axis=1)
    crw = np.concatenate([wstar[0:P, :], wstar[P:D, :], ident], axis=1)
    cf = np.concatenate(
        [bstar_t, a_t, np.zeros((P, PAD + 1), np.float32)], axis=1)

    shared = {
        "constsru": np.ascontiguousarray(cru.astype(BF16NP)),
        "constsrw": np.ascontiguousarray(crw.astype(BF16NP)),
        "constsf": np.ascontiguousarray(cf),
    }
    return [
        {"xt": np.ascontiguousarray(x[b].T.astype(BF16NP)), **shared}
        for b in range(NCORES)
    ]


def kernel(x, weight, input_weight, bias, tau, ksize, _trace=False):
    assert int(ksize) == KSIZE
    nc = get_program()
    in_maps = make_in_maps(x, weight, input_weight, bias, tau)
    res = run_bass_kernel_spmd(
        nc, in_maps, core_ids=list(range(NCORES)), trace=_trace
    )
    out = np.stack(
        [np.ascontiguousarray(res.results[b]["out"].T) for b in range(NCORES)],
        axis=0,
    )
    if _trace:
        _cache["last_results"] = res
    return out.astype(np.float32)
